# revision 5
# baseline (speedup 1.0000x reference)
"""DiffTexture bilinear sampling kernel for TRN2 (8 NeuronCores).

Strategy (data-parallel over sample points, texture replicated):
  - Each core handles N/8 = 1,048,576 points.
  - Phase 1 (per core): build a 2x2-block table B in DRAM:
      B[u, v] = [T[u,v], T[u,v+1], T[u+1,v], T[u+1,v+1]]  (12 f32 = 48B)
    for u in [0, 2046] (col 2047 garbage, never read), via dense DMA
    loads + DVE strided interleave copies.  While B is building, the
    otherwise-idle Pool engine runs a PROLOGUE: the first 32K points are
    gathered straight from the raw texture (two 24B indirect DMAs per
    128-point chunk - no B dependency), hiding the build latency.
  - Phase 2: per 128-point chunk, one indirect DMA gathers each point's
    48B block (idx = u0a*2048 + v0a); bilinear blend with adjusted
    weights folds the floor/ceil edge cases into the lerp weights:
      WU = a*mu0 + (1-a)*mu1, mu_i = (row_i != u0a)  (same for WV)
      out = lerp(lerp(p00,p01,WV), lerp(p10,p11,WV), WU); tanh on ACT.

Floor is built from the DVE round-to-nearest f32->i32 cast plus a
compare fix-up (no floor ALU op on TRN2).
"""

import numpy as np

import concourse.bass as bass
import concourse.bacc as bacc
import concourse.mybir as mybir
from concourse import tile
from concourse.bass_utils import run_bass_kernel_spmd

H = 2048
W = 2048
N_FULL = 8388608
NCORES = 8
P = 128
K = 512                  # points per partition per macro-tile
TPOINTS = P * K          # 65536 points per macro-tile
KPRO = 256               # prologue columns of tile 0 (raw-texture path)

f32 = mybir.dt.float32
i32 = mybir.dt.int32

ROW = W * 3              # texture row, elements (6144)
BROW = W * 12            # block-table row, elements (24576)
BROWS = H - 1            # block-table rows built (2047)

ALU = mybir.AluOpType


def _ap(t_ap, extra_offset, dims):
    """Build a raw AP on the same tensor as t_ap with given dims."""
    return bass.AP(t_ap.tensor, t_ap.offset + extra_offset, dims)


def _coord(nc, pool, src_ap, name, kk, bufs=None):
    """Pixel-space coords: returns (weight-of-+1-cell, clamped low index)."""
    cu = pool.tile([P, kk], f32, tag=f"{name}_cu", bufs=bufs)
    nc.vector.tensor_scalar(out=cu[:], in0=src_ap, scalar1=1.0, scalar2=0.5,
                            op0=ALU.add, op1=ALU.mult)
    nc.vector.tensor_scalar(out=cu[:], in0=cu[:], scalar1=float(W - 1),
                            scalar2=None, op0=ALU.mult)
    ci = pool.tile([P, kk], i32, tag=f"{name}_ci", bufs=bufs)
    nc.vector.tensor_copy(ci[:], cu[:])
    t1 = pool.tile([P, kk], f32, tag=f"{name}_t1", bufs=bufs)
    nc.vector.tensor_copy(t1[:], ci[:])                  # rcf = rint(u)
    t2 = pool.tile([P, kk], f32, tag=f"{name}_t2", bufs=bufs)
    nc.vector.tensor_tensor(out=t2[:], in0=t1[:], in1=cu[:], op=ALU.subtract)
    nc.vector.tensor_scalar(out=t2[:], in0=t2[:], scalar1=0.0, scalar2=0.0,
                            op0=ALU.max, op1=ALU.not_equal)  # (rcf>u)
    nc.vector.tensor_tensor(out=t1[:], in0=t1[:], in1=t2[:], op=ALU.subtract)
    fr = pool.tile([P, kk], f32, tag=f"{name}_fr", bufs=bufs)
    nc.vector.tensor_tensor(out=fr[:], in0=cu[:], in1=t1[:], op=ALU.subtract)
    nc.vector.tensor_tensor(out=t2[:], in0=cu[:], in1=t1[:], op=ALU.not_equal)
    nc.vector.tensor_tensor(out=cu[:], in0=t1[:], in1=t2[:], op=ALU.add)  # i1f
    i0af = pool.tile([P, kk], f32, tag=f"{name}_i0af", bufs=bufs)
    nc.vector.tensor_scalar(out=i0af[:], in0=t1[:], scalar1=float(W - 2),
                            scalar2=None, op0=ALU.min)
    nc.vector.tensor_tensor(out=t1[:], in0=t1[:], in1=i0af[:],
                            op=ALU.not_equal)            # m0
    nc.vector.tensor_tensor(out=t2[:], in0=cu[:], in1=i0af[:],
                            op=ALU.not_equal)            # m1
    nc.vector.tensor_tensor(out=t1[:], in0=t1[:], in1=t2[:], op=ALU.subtract)
    nc.vector.tensor_tensor(out=t1[:], in0=t1[:], in1=fr[:], op=ALU.mult)
    wt = pool.tile([P, kk], f32, tag=f"{name}_wt", bufs=bufs)
    nc.vector.tensor_tensor(out=wt[:], in0=t1[:], in1=t2[:], op=ALU.add)
    return wt, i0af


def _blend_store(nc, pool, patch, wu, wv, out_dst, kk, bufs=None):
    """patch [P, 12*kk] = [c00,c01,c10,c11]*kk -> tanh(bilinear) -> out."""
    pap = patch[:]
    p00 = _ap(pap, 0, [pap.ap[0], [12, kk], [1, 3]])
    p01 = _ap(pap, 3, [pap.ap[0], [12, kk], [1, 3]])
    p10 = _ap(pap, 6, [pap.ap[0], [12, kk], [1, 3]])
    p11 = _ap(pap, 9, [pap.ap[0], [12, kk], [1, 3]])
    wv3 = pool.tile([P, 3 * kk], f32, tag="wv3", bufs=bufs)
    wu3 = pool.tile([P, 3 * kk], f32, tag="wu3", bufs=bufs)
    for ch in range(3):
        nc.vector.tensor_copy(
            _ap(wv3[:], ch, [wv3[:].ap[0], [3, kk], [1, 1]]),
            _ap(wv[:], 0, [wv[:].ap[0], [1, kk], [1, 1]]))
        nc.vector.tensor_copy(
            _ap(wu3[:], ch, [wu3[:].ap[0], [3, kk], [1, 1]]),
            _ap(wu[:], 0, [wu[:].ap[0], [1, kk], [1, 1]]))
    wvb = _ap(wv3[:], 0, [wv3[:].ap[0], [3, kk], [1, 3]])
    wub = _ap(wu3[:], 0, [wu3[:].ap[0], [3, kk], [1, 3]])

    def v3(t):
        return _ap(t[:], 0, [t[:].ap[0], [3, kk], [1, 3]])

    r0 = pool.tile([P, 3 * kk], f32, tag="r0", bufs=bufs)
    r1 = pool.tile([P, 3 * kk], f32, tag="r1", bufs=bufs)
    res = pool.tile([P, 3 * kk], f32, tag="res", bufs=bufs)
    nc.vector.tensor_tensor(out=v3(r0), in0=p01, in1=p00, op=ALU.subtract)
    nc.vector.tensor_tensor(out=v3(r0), in0=v3(r0), in1=wvb, op=ALU.mult)
    nc.vector.tensor_tensor(out=v3(r0), in0=v3(r0), in1=p00, op=ALU.add)
    nc.vector.tensor_tensor(out=v3(r1), in0=p11, in1=p10, op=ALU.subtract)
    nc.vector.tensor_tensor(out=v3(r1), in0=v3(r1), in1=wvb, op=ALU.mult)
    nc.vector.tensor_tensor(out=v3(r1), in0=v3(r1), in1=p10, op=ALU.add)
    # res = r0 + WU*(r1-r0)   (WU = weight of the +1 row)
    nc.vector.tensor_tensor(out=v3(res), in0=v3(r1), in1=v3(r0),
                            op=ALU.subtract)
    nc.vector.tensor_tensor(out=v3(res), in0=v3(res), in1=wub, op=ALU.mult)
    nc.vector.tensor_tensor(out=v3(res), in0=v3(res), in1=v3(r0), op=ALU.add)
    nc.scalar.activation(out=res[:], in_=res[:],
                         func=mybir.ActivationFunctionType.Tanh)
    nc.sync.dma_start(out=out_dst, in_=res[:])


def build_nc(npc):
    """Build the per-core Bass program for npc points (npc % TPOINTS == 0)."""
    ntiles = npc // TPOINTS
    nc = bacc.Bacc("TRN2", target_bir_lowering=False)

    uvs = nc.dram_tensor("uvs", [npc, 2], f32, kind="ExternalInput")
    texture = nc.dram_tensor("texture", [H, W, 3], f32, kind="ExternalInput")
    out = nc.dram_tensor("out", [npc, 3], f32, kind="ExternalOutput")
    btab = nc.dram_tensor("btab", [BROWS * W, 12], f32)  # internal, 192MB

    tex_flat = texture[:].rearrange("h w c -> (h w c)")
    tex2 = texture[:].rearrange("h w c -> (h w) c")
    uvs_t = uvs[:].rearrange("(t p k) c -> t p (k c)", t=ntiles, p=P, k=K)
    out_t = out[:].rearrange("(t p k) c -> t p (k c)", t=ntiles, p=P, k=K)

    with tile.TileContext(nc) as tc:
        # ---- Phase 1: build the 2x2 block table + raw-texture prologue ---
        with tc.tile_pool(name="bpool", bufs=2) as bp:
            for blk in range(16):
                u0 = blk * 128
                nr = 128 if blk < 15 else 127          # rows this block
                a_t = bp.tile([P, ROW + 3], f32, tag="arow")
                a1_t = bp.tile([P, ROW + 3], f32, tag="a1row")
                nc.sync.dma_start(
                    out=a_t[:nr, :],
                    in_=_ap(tex_flat, u0 * ROW, [[ROW, nr], [1, ROW + 3]]))
                a1_len = ROW + 3 if blk < 15 else ROW
                nc.sync.dma_start(
                    out=a1_t[:nr, :a1_len],
                    in_=_ap(tex_flat, (u0 + 1) * ROW, [[ROW, nr], [1, a1_len]]))
                for c in range(4):      # four 512-column chunks
                    bt = bp.tile([P, 12 * 512], f32, tag="bchunk")
                    voff = c * 512 * 3
                    for (dst_off, src, src_off) in (
                        (0, a_t, 0), (3, a_t, 3), (6, a1_t, 0), (9, a1_t, 3),
                    ):
                        nc.vector.tensor_copy(
                            _ap(bt[:], dst_off, [bt[:].ap[0], [12, 512], [1, 3]]),
                            _ap(src[:], voff + src_off,
                                [src[:].ap[0], [3, 512], [1, 3]]))
                    nc.sync.dma_start(
                        out=_ap(btab[:], u0 * BROW + c * 12 * 512,
                                [[BROW, nr], [1, 12 * 512]]),
                        in_=bt[:nr, :])

            # Prologue: first KPRO columns of tile 0 via raw-texture gathers
            # (runs on Pool while the B build occupies DMA + DVE).
            uv0 = bp.tile([P, 2 * KPRO], f32, tag="puv", bufs=1)
            nc.sync.dma_start(out=uv0[:], in_=uvs_t[0][:, :2 * KPRO])
            px = _ap(uv0[:], 0, [uv0[:].ap[0], [2, KPRO]])
            py = _ap(uv0[:], 1, [uv0[:].ap[0], [2, KPRO]])
            pwu, pu0af = _coord(nc, bp, px, "pu", KPRO, bufs=1)
            pwv, pv0af = _coord(nc, bp, py, "pv", KPRO, bufs=1)
            pidxf = bp.tile([P, KPRO], f32, tag="pidxf", bufs=1)
            nc.vector.scalar_tensor_tensor(
                out=pidxf[:], in0=pu0af[:], scalar=float(W), in1=pv0af[:],
                op0=ALU.mult, op1=ALU.add)
            pidx0 = bp.tile([P, KPRO], i32, tag="pidx0", bufs=1)
            nc.vector.tensor_copy(pidx0[:], pidxf[:])
            nc.vector.tensor_scalar(out=pidxf[:], in0=pidxf[:],
                                    scalar1=float(W), scalar2=None,
                                    op0=ALU.add)
            pidx1 = bp.tile([P, KPRO], i32, tag="pidx1", bufs=1)
            nc.vector.tensor_copy(pidx1[:], pidxf[:])
            ppatch = bp.tile([P, 12 * KPRO], f32, tag="ppatch", bufs=1)
            for k in range(KPRO):
                nc.gpsimd.indirect_dma_start(
                    out=ppatch[:, 12 * k:12 * k + 6],
                    out_offset=None, in_=tex2,
                    in_offset=bass.IndirectOffsetOnAxis(
                        ap=pidx0[:, k:k + 1], axis=0))
                nc.gpsimd.indirect_dma_start(
                    out=ppatch[:, 12 * k + 6:12 * k + 12],
                    out_offset=None, in_=tex2,
                    in_offset=bass.IndirectOffsetOnAxis(
                        ap=pidx1[:, k:k + 1], axis=0))
            _blend_store(nc, bp, ppatch, pwu, pwv,
                         out_t[0][:, :3 * KPRO], KPRO, bufs=1)

        tc.strict_bb_all_engine_barrier()

        # ---- Phase 2: per-tile sample via the block table ----------------
        with tc.tile_pool(name="main", bufs=2) as mp:
            for ti in range(ntiles):
                koff = KPRO if ti == 0 else 0
                kk = K - koff
                uv = mp.tile([P, 2 * kk], f32, tag="uv")
                nc.sync.dma_start(out=uv[:],
                                  in_=uvs_t[ti][:, 2 * koff:2 * K])
                x_ap = _ap(uv[:], 0, [uv[:].ap[0], [2, kk]])
                y_ap = _ap(uv[:], 1, [uv[:].ap[0], [2, kk]])
                wu, u0af = _coord(nc, mp, x_ap, "u", kk)
                wv, v0af = _coord(nc, mp, y_ap, "v", kk)
                idxf = mp.tile([P, kk], f32, tag="idxf")
                nc.vector.scalar_tensor_tensor(
                    out=idxf[:], in0=u0af[:], scalar=float(W), in1=v0af[:],
                    op0=ALU.mult, op1=ALU.add)
                idx = mp.tile([P, kk], i32, tag="idx")
                nc.vector.tensor_copy(idx[:], idxf[:])
                patch = mp.tile([P, 12 * kk], f32, tag="patch")
                for k in range(kk):
                    nc.gpsimd.indirect_dma_start(
                        out=patch[:, 12 * k:12 * (k + 1)],
                        out_offset=None, in_=btab[:],
                        in_offset=bass.IndirectOffsetOnAxis(
                            ap=idx[:, k:k + 1], axis=0))
                _blend_store(nc, mp, patch, wu, wv,
                             out_t[ti][:, 3 * koff:3 * K], kk)

    nc.compile()
    return nc


_NC_CACHE = {}


def _get_nc(npc):
    if npc not in _NC_CACHE:
        _NC_CACHE[npc] = build_nc(npc)
    return _NC_CACHE[npc]


def kernel(uvs, texture):
    uvs = np.ascontiguousarray(uvs, dtype=np.float32)
    texture = np.ascontiguousarray(texture, dtype=np.float32)
    assert uvs.shape == (N_FULL, 2) and texture.shape == (H, W, 3)
    npc = N_FULL // NCORES
    nc = _get_nc(npc)
    in_maps = [
        {"uvs": uvs[c * npc:(c + 1) * npc], "texture": texture}
        for c in range(NCORES)
    ]
    res = run_bass_kernel_spmd(nc, in_maps, core_ids=list(range(NCORES)))
    return np.concatenate([r["out"] for r in res.results], axis=0)
